# revision 5
# baseline (speedup 1.0000x reference)
"""Trainium2 Bass kernel for pairwise-MLP GNN message passing.

dro[b,i,j] = w3^T relu(W2^T relu(hA_i + hB_j) + b2) + b3, with the first
linear layer factorized as hA_i + hB_j (no relu between concat and W1).

Sharding: robot-row dimension N=512 split across 8 cores (64 rows each).

v3 engine split (per robot row i), all three engines ~equally loaded:
  L1 (t1_k = relu(hBT_k + hA[:,i])): k-tiles 0,1 on DVE (bf16 4x map,
     ~264ns each), k-tile 2 on ACT (activation+bias, ~630ns).
  L2 on PE (bf16): z2[jt] = t1^T @ W2p with |w3| folded into W2 columns
     (columns permuted pos|neg). K-tiles {128,128,64}; the K=64 tails of
     adjacent jt pairs run concurrently in row groups (0,0)/(64,0):
     10 x 320-cycle slots per i.
  L3 (osig[j,i] = sum_h sign_h relu(z2[j,h])):
     jt0 on ACT: two relu+accum_out ops (pos block / neg block, ~576ns
     each), combined P-N at the epilogue.
     jt1..3 on DVE: fused scalar_tensor_tensor max(z2,0)*sign with
     accum_out straight from PSUM (~403ns each).
  Output is stored j-major [B, N, NI]; host transposes and adds b3.
"""

import numpy as np
import ml_dtypes

import concourse.bass as bass
import concourse.mybir as mybir
import concourse.tile as tile
from concourse import bacc
from concourse import bass_utils
from concourse.masks import make_identity

F32 = mybir.dt.float32
F32R = mybir.dt.float32r
BF16 = mybir.dt.bfloat16
ALU = mybir.AluOpType
ACTF = mybir.ActivationFunctionType

B, N, E, L = 2, 512, 128, 32
D = E + L            # 160
H = 2 * D            # 320
NCORES = 8
NI = N // NCORES     # 64 robot rows per core
NJT = 4              # j-tiles of 128
MS = [(0, 128), (128, 128), (256, 64)]   # m-tiles of H (hA/hB build)

_CACHE = {}


def _build(hp, with_bias):
    """hp: size of the positive-w3 column block (columns permuted pos|neg)."""
    kc = 65 if with_bias else 64      # third k-tile height (64 data + ones)
    act_jt0 = 0 < hp < H              # jt0 L3 via ACT +-accum pair

    nc = bacc.Bacc("TRN2", target_bir_lowering=False, debug=False,
                   enable_asserts=False, num_devices=NCORES)

    robot = nc.dram_tensor("robot", [B, NI, E], F32, kind="ExternalInput").ap()
    obj = nc.dram_tensor("obj", [B, N, E], F32, kind="ExternalInput").ap()
    W1A = nc.dram_tensor("W1A", [E, H], F32, kind="ExternalInput").ap()
    W1B = nc.dram_tensor("W1B", [E, H], F32, kind="ExternalInput").ap()
    zAT = nc.dram_tensor("zAT", [H, B], F32, kind="ExternalInput").ap()
    zBT = nc.dram_tensor("zBT", [H, B], F32, kind="ExternalInput").ap()
    w2a = nc.dram_tensor("w2a", [128, H], BF16, kind="ExternalInput").ap()
    w2b = nc.dram_tensor("w2b", [128, H], BF16, kind="ExternalInput").ap()
    # k rows 256:320 (+ones row if biased); duplicated at partitions 64+
    # in the unbiased case so the K=64 tails can pair in row groups.
    w2c = nc.dram_tensor("w2c", [kc if with_bias else 128, H], BF16,
                         kind="ExternalInput").ap()
    signsd = nc.dram_tensor("signs", [128, H], F32, kind="ExternalInput").ap()
    out = nc.dram_tensor("out", [B, N, NI], F32, kind="ExternalOutput").ap()

    with tile.TileContext(nc) as tc:
        with tc.tile_pool(name="persist", bufs=1) as pp:
            ident = pp.tile([128, 128], F32, tag="ident")
            make_identity(nc, ident[:])
            # force the ACT Relu table load early so it overlaps setup
            warm = pp.tile([1, 1], F32, tag="warm")
            nc.scalar.activation(warm[:], ident[0:1, 0:1], ACTF.Relu)
            sg = pp.tile([128, H], F32, tag="sg")
            nc.scalar.dma_start(sg[:], signsd)

            # ---- weight tiles (bf16 direct from HBM) ----
            w2at = pp.tile([128, H], BF16, tag="w2a")
            nc.sync.dma_start(w2at[:], w2a)
            w2bt = pp.tile([128, H], BF16, tag="w2b")
            nc.scalar.dma_start(w2bt[:], w2b)
            w2ct = pp.tile([kc if with_bias else 128, H], BF16, tag="w2c")
            nc.sync.dma_start(w2ct[:], w2c)

            # f32r W1 halves for the setup matmuls
            with tc.tile_pool(name="wstg", bufs=2) as wstg:
                stg = wstg.tile([E, H], F32, tag="wstg")
                nc.sync.dma_start(stg[:], W1A)
                w1a = pp.tile([E, H], F32R, tag="w1a")
                nc.vector.tensor_copy(w1a[:], stg[:])
                stg = wstg.tile([E, H], F32, tag="wstg")
                nc.scalar.dma_start(stg[:], W1B)
                w1b = pp.tile([E, H], F32R, tag="w1b")
                nc.vector.tensor_copy(w1b[:], stg[:])

            zat, zbt = [], []
            for m, (m0, sz) in enumerate(MS):
                t = pp.tile([sz, B], F32, tag=f"zat_{m}")
                nc.sync.dma_start(t[:], zAT[m0:m0 + sz, :])
                zat.append(t)
                t = pp.tile([sz, B], F32, tag=f"zbt_{m}")
                nc.sync.dma_start(t[:], zBT[m0:m0 + sz, :])
                zbt.append(t)

            hbt = {}  # (b, k) -> bf16 tile: [128, N] (k=2 dup'd rows 64+)
            hat = {}  # (b, k) -> f32 tile: [128, NI] (k=2 dup'd)

            # ---- setup: build hA^T, hB^T on device (bf16 outputs) ----
            with tc.tile_pool(name="s_sb", bufs=2) as ssb, \
                 tc.tile_pool(name="s_ps", bufs=2, space="PSUM") as sps:
                for b in range(B):
                    # hB^T[b]: [H, N] = W1B^T @ obj[b]^T (+ zB bias)
                    objT_ps = sps.tile([128, N], F32, tag="objT_ps")
                    for jt in range(NJT):
                        stg = ssb.tile([128, E], F32, tag="stg", bufs=2)
                        qs = ([nc.sync, nc.scalar, nc.sync, nc.scalar]
                              if b == 0 else
                              [nc.gpsimd, nc.gpsimd, nc.gpsimd, nc.gpsimd])
                        qs[jt].dma_start(
                            stg[:], obj[b, jt * 128:(jt + 1) * 128, :])
                        nc.tensor.transpose(objT_ps[:, jt * 128:(jt + 1) * 128],
                                            stg[:], ident[:])
                    objT = ssb.tile([128, N], F32R, tag="objT")
                    nc.vector.tensor_copy(objT[:], objT_ps[:])
                    for m, (m0, sz) in enumerate(MS):
                        hps = sps.tile([sz, N], F32, tag="hps")
                        nc.tensor.matmul(hps[:], w1b[:, m0:m0 + sz], objT[:],
                                         start=True, stop=True)
                        t = pp.tile([128, N] if m == 2 else [sz, N], BF16,
                                    tag=f"hbt_{b}_{m}")
                        nc.vector.tensor_scalar(
                            out=t[0:sz, :], in0=hps[:],
                            scalar1=zbt[m][:, b:b + 1],
                            scalar2=None, op0=ALU.add)
                        if m == 2:
                            if with_bias:
                                nc.gpsimd.memset(t[64:65, :], 1.0)
                            else:
                                # duplicate k rows 256:320 at partitions
                                # 64:128 for the paired K=64 matmul tails
                                nc.gpsimd.dma_start(t[64:128, :], t[0:64, :])
                        hbt[(b, m)] = t

                    # hA^T[b]: [H, NI] from robot[b] @ W1A (+ zA bias)
                    stg2 = ssb.tile([NI, E], F32, tag="stg2")
                    (nc.scalar if b == 0 else nc.gpsimd).dma_start(
                        stg2[:], robot[b, :, :])
                    robT_ps = sps.tile([128, NI], F32, tag="robT_ps")
                    nc.tensor.transpose(robT_ps[:], stg2[:], ident[0:NI, 0:NI])
                    robT = ssb.tile([128, NI], F32R, tag="robT")
                    nc.vector.tensor_copy(robT[:], robT_ps[:])
                    for m, (m0, sz) in enumerate(MS):
                        aps_ = sps.tile([sz, NI], F32, tag="aps")
                        nc.tensor.matmul(aps_[:], w1a[:, m0:m0 + sz], robT[:],
                                         start=True, stop=True)
                        t = pp.tile([128, NI] if m == 2 else [sz, NI], F32,
                                    tag=f"hat_{b}_{m}")
                        nc.vector.tensor_scalar(
                            out=t[0:sz, :], in0=aps_[:],
                            scalar1=zat[m][:, b:b + 1],
                            scalar2=None, op0=ALU.add)
                        if m == 2:
                            if with_bias:
                                nc.gpsimd.memset(t[64:65, :], 0.0)
                            else:
                                nc.gpsimd.dma_start(t[64:128, :], t[0:64, :])
                        hat[(b, m)] = t

            # ---- main loop ----
            with tc.tile_pool(name="t1p", bufs=2) as t1p, \
                 tc.tile_pool(name="z2p", bufs=2, space="PSUM") as z2p, \
                 tc.tile_pool(name="scr", bufs=2) as scr, \
                 tc.tile_pool(name="accp", bufs=2) as accp:
                for b in range(B):
                    osig = {jt: accp.tile([128, NI], F32, tag=f"os_{jt}",
                                          name=f"os_{jt}_{b}")
                            for jt in range(1, NJT)}
                    osP = accp.tile([128, NI], F32, tag="osP", name=f"osP_{b}")
                    osN = accp.tile([128, NI], F32, tag="osN", name=f"osN_{b}")
                    prev = None

                    def emit_l3(quad, i):
                        if act_jt0:
                            so = scr.tile([128, H], BF16, tag="scrA")
                            nc.scalar.activation(
                                so[:, 0:hp], quad[:, 0, 0:hp], ACTF.Relu,
                                accum_out=osP[:, i:i + 1])
                            so = scr.tile([128, H], BF16, tag="scrB")
                            nc.scalar.activation(
                                so[:, 0:H - hp], quad[:, 0, hp:H], ACTF.Relu,
                                accum_out=osN[:, i:i + 1])
                            jts = range(1, NJT)
                        else:
                            jts = range(NJT)
                        for jt in jts:
                            sv = scr.tile([128, H], F32, tag=f"scrV{jt % 2}")
                            tgt = osig[jt] if jt in osig else osP
                            nc.vector.scalar_tensor_tensor(
                                out=sv[:], in0=quad[:, jt, 0:H], scalar=0.0,
                                in1=sg[:], op0=ALU.max, op1=ALU.mult,
                                accum_out=tgt[:, i:i + 1])

                    for i in range(NI):
                        # L1: t1_k = relu(hBT_k + hA_col)
                        t1 = []
                        for k in range(2):
                            t = t1p.tile([128, N], BF16, tag=f"t1_{k}")
                            nc.vector.tensor_scalar(
                                out=t[:], in0=hbt[(b, k)][:],
                                scalar1=hat[(b, k)][:, i:i + 1],
                                scalar2=0.0, op0=ALU.add, op1=ALU.max)
                            t1.append(t)
                        kp = kc if with_bias else 128
                        t = t1p.tile([kp, N], BF16, tag="t1_2")
                        nc.scalar.activation(
                            t[:], hbt[(b, 2)][0:kp, :], ACTF.Relu,
                            bias=hat[(b, 2)][0:kp, i:i + 1])
                        t1.append(t)

                        # L2: z2[jt] = t1^T[jt] @ W2p on PE (bf16)
                        quad = z2p.tile([128, NJT, 512], F32, tag="quad")
                        for half in range(2):
                            jts = (0, 1) if half == 0 else (2, 3)
                            for jt in jts:
                                js = slice(jt * 128, (jt + 1) * 128)
                                nc.tensor.matmul(
                                    quad[:, jt, 0:H], t1[0][:, js], w2at[:],
                                    start=True, stop=False)
                                nc.tensor.matmul(
                                    quad[:, jt, 0:H], t1[1][:, js], w2bt[:],
                                    start=False, stop=False)
                            if with_bias:
                                for jt in jts:
                                    js = slice(jt * 128, (jt + 1) * 128)
                                    nc.tensor.matmul(
                                        quad[:, jt, 0:H], t1[2][0:kc, js],
                                        w2ct[0:kc, :],
                                        start=False, stop=True)
                            else:
                                # K=64 tails of the jt pair run concurrently
                                # in row groups (0,0) and (64,0)
                                jt0, jt1 = jts
                                js0 = slice(jt0 * 128, (jt0 + 1) * 128)
                                js1 = slice(jt1 * 128, (jt1 + 1) * 128)
                                nc.tensor.matmul(
                                    quad[:, jt0, 0:H], t1[2][0:64, js0],
                                    w2ct[0:64, :], start=False, stop=True)
                                nc.tensor.matmul(
                                    quad[:, jt1, 0:H], t1[2][64:128, js1],
                                    w2ct[64:128, :], start=False, stop=True)

                        # L3 for the previous i (software pipeline: keeps
                        # both engines fed with L1(i) first)
                        if prev is not None:
                            emit_l3(prev[0], prev[1])
                        prev = (quad, i)

                    emit_l3(prev[0], prev[1])

                    # epilogue: store j-major [N, NI] slabs
                    if act_jt0:
                        osb = accp.tile([128, NI], F32, tag="osb",
                                        name=f"osb_{b}")
                        nc.vector.scalar_tensor_tensor(
                            out=osb[:], in0=osP[:], scalar=0.0,
                            in1=osN[:], op0=ALU.add, op1=ALU.subtract)
                        nc.sync.dma_start(out[b, 0:128, :], osb[:])
                    else:
                        nc.sync.dma_start(out[b, 0:128, :], osP[:])
                    for jt in range(1, NJT):
                        nc.sync.dma_start(
                            out[b, jt * 128:(jt + 1) * 128, :], osig[jt][:])

    nc.compile()
    return nc


def _prep(robot_embedding_tf, object_embedding_tf, z, W1, b1, W2, b2, W3, b3):
    """Host-side weight prep (O(H^2)) + per-core input maps."""
    f = np.float32
    bf = ml_dtypes.bfloat16
    robot = np.ascontiguousarray(robot_embedding_tf, dtype=f)
    obj = np.ascontiguousarray(object_embedding_tf, dtype=f)
    z = np.asarray(z, dtype=f)
    W1 = np.asarray(W1, dtype=f)
    b1 = np.asarray(b1, dtype=f)
    W2 = np.asarray(W2, dtype=f)
    b2 = np.asarray(b2, dtype=f)
    W3 = np.asarray(W3, dtype=f)
    b3 = np.asarray(b3, dtype=f)

    w3 = W3[:, 0]
    aw3 = np.abs(w3)
    pos = np.nonzero(w3 >= 0)[0]
    neg = np.nonzero(w3 < 0)[0]
    hp = len(pos)
    perm = np.concatenate([pos, neg])

    with_bias = bool(np.any(b2))
    kc = 65 if with_bias else 64
    W2p = (W2 * aw3[None, :])[:, perm]
    b2p = (b2 * aw3)[perm]
    W2cols = np.concatenate([W2p, b2p[None, :]], axis=0)  # [H+1, H]
    signs = np.zeros((128, H), dtype=f)
    signs[:, 0:hp] = 1.0
    signs[:, hp:H] = -1.0

    w2a_ = np.ascontiguousarray(W2cols[0:128], dtype=bf)
    w2b_ = np.ascontiguousarray(W2cols[128:256], dtype=bf)
    if with_bias:
        w2c_ = np.ascontiguousarray(W2cols[256:256 + kc], dtype=bf)
    else:
        w2c_ = np.ascontiguousarray(
            np.concatenate([W2cols[256:320], W2cols[256:320]], axis=0),
            dtype=bf)

    zA = z @ W1[E:D, :]                 # [B, H]
    zB = z @ W1[D + E:, :] + b1[None, :]
    zAT = np.ascontiguousarray(zA.T, dtype=f)
    zBT = np.ascontiguousarray(zB.T, dtype=f)
    W1A = np.ascontiguousarray(W1[0:E, :], dtype=f)
    W1B = np.ascontiguousarray(W1[D:D + E, :], dtype=f)

    shared = dict(obj=obj, W1A=W1A, W1B=W1B, zAT=zAT, zBT=zBT,
                  w2a=w2a_, w2b=w2b_, w2c=w2c_, signs=signs)
    in_maps = []
    for c in range(NCORES):
        m = dict(shared)
        m["robot"] = np.ascontiguousarray(robot[:, c * NI:(c + 1) * NI, :])
        in_maps.append(m)
    return in_maps, (hp, with_bias), float(b3[0])


def _run(trace=False, **inputs):
    in_maps, key, b3v = _prep(**inputs)
    if key not in _CACHE:
        _CACHE[key] = _build(*key)
    nc = _CACHE[key]
    res = bass_utils.run_bass_kernel_spmd(
        nc, in_maps, core_ids=list(range(NCORES)), trace=trace)
    dro = np.empty((B, N, N), dtype=np.float32)
    for c in range(NCORES):
        # device output is j-major [B, N, NI]; transpose to [B, NI, N]
        dro[:, c * NI:(c + 1) * NI, :] = np.transpose(
            res.results[c]["out"], (0, 2, 1))
    if b3v != 0.0:
        dro += b3v
    return dro, res


def kernel(**inputs) -> np.ndarray:
    dro, _ = _run(trace=False, **inputs)
    return dro


# revision 8
# speedup vs baseline: 1.4603x; 1.4603x over previous
"""Trainium2 Bass kernel for pairwise-MLP GNN message passing.

dro[b,i,j] = w3^T relu(W2^T relu(hA_i + hB_j) + b2) + b3, with the first
linear layer factorized as hA_i + hB_j (no relu between concat and W1).

Sharding: robot-row dimension N=512 split across 8 cores (64 rows each).

v4 (per robot row i):
  L1 on ACT (activation Relu with per-partition bias, ~630ns x3): t1_k =
     relu(hBT_k + hA[:,i]) in bf16, emitted one step ahead of the PE.
  L2 on PE (bf16): z2[jt] = t1^T @ W2p (|w3| folded into W2 columns).
     K-tiles {128,128,64}; the K=64 tails of adjacent jt pairs run
     concurrently in row groups (0,0)/(64,0): 10 x 320-cycle slots/i.
  L3 on DVE: per jt one fused scalar_tensor_tensor max(z2,0)*sign with
     accum_out straight from PSUM (~450ns x4). Per-jt PSUM tiles so each
     stt frees its bank for the PE independently.
  Output stored j-major [B, N, NI]; host transposes and adds b3.
"""

import numpy as np
import ml_dtypes

import concourse.bass as bass
import concourse.mybir as mybir
import concourse.tile as tile
from concourse import bacc
from concourse import bass_utils
from concourse.masks import make_identity

F32 = mybir.dt.float32
F32R = mybir.dt.float32r
BF16 = mybir.dt.bfloat16
ALU = mybir.AluOpType
ACTF = mybir.ActivationFunctionType

B, N, E, L = 2, 512, 128, 32
D = E + L            # 160
H = 2 * D            # 320
NCORES = 8
NI = N // NCORES     # 64 robot rows per core
NJT = 4              # j-tiles of 128
MS = [(0, 128), (128, 128), (256, 64)]   # m-tiles of H (hA/hB build)

_CACHE = {}


def _build(with_bias):
    kc = 65 if with_bias else 64      # third k-tile height (64 data + ones)

    nc = bacc.Bacc("TRN2", target_bir_lowering=False, debug=False,
                   enable_asserts=False, num_devices=NCORES)

    robot = nc.dram_tensor("robot", [B, NI, E], F32, kind="ExternalInput").ap()
    obj = nc.dram_tensor("obj", [B, N, E], F32, kind="ExternalInput").ap()
    W1A = nc.dram_tensor("W1A", [E, H], F32, kind="ExternalInput").ap()
    W1B = nc.dram_tensor("W1B", [E, H], F32, kind="ExternalInput").ap()
    zAT = nc.dram_tensor("zAT", [H, B], F32, kind="ExternalInput").ap()
    zBT = nc.dram_tensor("zBT", [H, B], F32, kind="ExternalInput").ap()
    w2a = nc.dram_tensor("w2a", [128, H], BF16, kind="ExternalInput").ap()
    w2b = nc.dram_tensor("w2b", [128, H], BF16, kind="ExternalInput").ap()
    # k rows 256:320 (+ones row if biased); duplicated at partitions 64+
    # in the unbiased case so the K=64 tails can pair in row groups.
    w2c = nc.dram_tensor("w2c", [kc if with_bias else 128, H], BF16,
                         kind="ExternalInput").ap()
    signsd = nc.dram_tensor("signs", [128, H], F32, kind="ExternalInput").ap()
    out = nc.dram_tensor("out", [B, N, NI], F32, kind="ExternalOutput").ap()

    with tile.TileContext(nc) as tc:
        with tc.tile_pool(name="persist", bufs=1) as pp:
            ident = pp.tile([128, 128], F32, tag="ident")
            make_identity(nc, ident[:])
            # force the ACT Relu table load early so it overlaps setup
            warm = pp.tile([1, 1], F32, tag="warm")
            nc.scalar.activation(warm[:], ident[0:1, 0:1], ACTF.Relu)
            sg = pp.tile([128, H], F32, tag="sg")
            nc.scalar.dma_start(sg[:], signsd)

            # ---- weight tiles (bf16 direct from HBM) ----
            w2at = pp.tile([128, H], BF16, tag="w2a")
            nc.sync.dma_start(w2at[:], w2a)
            w2bt = pp.tile([128, H], BF16, tag="w2b")
            nc.scalar.dma_start(w2bt[:], w2b)
            w2ct = pp.tile([kc if with_bias else 128, H], BF16, tag="w2c")
            nc.sync.dma_start(w2ct[:], w2c)

            # f32r W1 halves for the setup matmuls
            with tc.tile_pool(name="wstg", bufs=2) as wstg:
                stg = wstg.tile([E, H], F32, tag="wstg")
                nc.sync.dma_start(stg[:], W1A)
                w1a = pp.tile([E, H], F32R, tag="w1a")
                nc.vector.tensor_copy(w1a[:], stg[:])
                stg = wstg.tile([E, H], F32, tag="wstg")
                nc.scalar.dma_start(stg[:], W1B)
                w1b = pp.tile([E, H], F32R, tag="w1b")
                nc.vector.tensor_copy(w1b[:], stg[:])

            zat, zbt = [], []
            for m, (m0, sz) in enumerate(MS):
                t = pp.tile([sz, B], F32, tag=f"zat_{m}")
                nc.sync.dma_start(t[:], zAT[m0:m0 + sz, :])
                zat.append(t)
                t = pp.tile([sz, B], F32, tag=f"zbt_{m}")
                nc.sync.dma_start(t[:], zBT[m0:m0 + sz, :])
                zbt.append(t)

            hbt = {}  # (b, k) -> bf16 tile: [128, N] (k=2 dup'd rows 64+)
            hat = {}  # (b, k) -> f32 tile: [128, NI] (k=2 dup'd)

            # ---- setup: build hA^T, hB^T on device (bf16 outputs) ----
            with tc.tile_pool(name="s_sb", bufs=2) as ssb, \
                 tc.tile_pool(name="s_ps", bufs=2, space="PSUM") as sps:
                for b in range(B):
                    # hB^T[b]: [H, N] = W1B^T @ obj[b]^T (+ zB bias)
                    objT_ps = sps.tile([128, N], F32, tag="objT_ps")
                    for jt in range(NJT):
                        stg = ssb.tile([128, E], F32, tag="stg", bufs=2)
                        qs = ([nc.sync, nc.scalar, nc.sync, nc.scalar]
                              if b == 0 else
                              [nc.gpsimd, nc.gpsimd, nc.gpsimd, nc.gpsimd])
                        qs[jt].dma_start(
                            stg[:], obj[b, jt * 128:(jt + 1) * 128, :])
                        nc.tensor.transpose(objT_ps[:, jt * 128:(jt + 1) * 128],
                                            stg[:], ident[:])
                    objT = ssb.tile([128, N], F32R, tag="objT")
                    nc.vector.tensor_copy(objT[:], objT_ps[:])
                    for m, (m0, sz) in enumerate(MS):
                        hps = sps.tile([sz, N], F32, tag="hps")
                        nc.tensor.matmul(hps[:], w1b[:, m0:m0 + sz], objT[:],
                                         start=True, stop=True)
                        t = pp.tile([128, N] if m == 2 else [sz, N], BF16,
                                    tag=f"hbt_{b}_{m}")
                        nc.vector.tensor_scalar(
                            out=t[0:sz, :], in0=hps[:],
                            scalar1=zbt[m][:, b:b + 1],
                            scalar2=None, op0=ALU.add)
                        if m == 2:
                            if with_bias:
                                nc.gpsimd.memset(t[64:65, :], 1.0)
                            else:
                                # duplicate k rows 256:320 at partitions
                                # 64:128 for the paired K=64 matmul tails
                                nc.gpsimd.dma_start(t[64:128, :], t[0:64, :])
                        hbt[(b, m)] = t

                    # hA^T[b]: [H, NI] from robot[b] @ W1A (+ zA bias)
                    stg2 = ssb.tile([NI, E], F32, tag="stg2")
                    (nc.scalar if b == 0 else nc.gpsimd).dma_start(
                        stg2[:], robot[b, :, :])
                    robT_ps = sps.tile([128, NI], F32, tag="robT_ps")
                    nc.tensor.transpose(robT_ps[:], stg2[:], ident[0:NI, 0:NI])
                    robT = ssb.tile([128, NI], F32R, tag="robT")
                    nc.vector.tensor_copy(robT[:], robT_ps[:])
                    for m, (m0, sz) in enumerate(MS):
                        aps_ = sps.tile([sz, NI], F32, tag="aps")
                        nc.tensor.matmul(aps_[:], w1a[:, m0:m0 + sz], robT[:],
                                         start=True, stop=True)
                        t = pp.tile([128, NI] if m == 2 else [sz, NI], F32,
                                    tag=f"hat_{b}_{m}")
                        nc.vector.tensor_scalar(
                            out=t[0:sz, :], in0=aps_[:],
                            scalar1=zat[m][:, b:b + 1],
                            scalar2=None, op0=ALU.add)
                        if m == 2:
                            if with_bias:
                                nc.gpsimd.memset(t[64:65, :], 0.0)
                            else:
                                nc.gpsimd.dma_start(t[64:128, :], t[0:64, :])
                        hat[(b, m)] = t

            # ---- main loop ----
            with tc.tile_pool(name="t1p", bufs=3) as t1p, \
                 tc.tile_pool(name="z2p", bufs=2, space="PSUM") as z2p, \
                 tc.tile_pool(name="scr", bufs=2) as scr, \
                 tc.tile_pool(name="accp", bufs=2) as accp:
                for b in range(B):
                    osig = {jt: accp.tile([128, NI], F32, tag=f"os_{jt}",
                                          name=f"os_{jt}_{b}")
                            for jt in range(NJT)}

                    def emit_t1(i):
                        """L1 for row i on ACT (bf16 out)."""
                        t1 = []
                        for k in range(3):
                            kp = 128 if (k < 2 or not with_bias) else kc
                            t = t1p.tile([kp, N], BF16, tag=f"t1_{k}")
                            nc.scalar.activation(
                                t[:], hbt[(b, k)][0:kp, :], ACTF.Relu,
                                bias=hat[(b, k)][0:kp, i:i + 1])
                            t1.append(t)
                        return t1

                    def emit_pe(t1, i):
                        """L2 for one row: 4 per-jt psum tiles."""
                        qs = [z2p.tile([128, 512], F32, tag=f"q{jt}",
                                       name=f"q{jt}_{b}_{i}")
                              for jt in range(NJT)]
                        for half in range(2):
                            jts = (0, 1) if half == 0 else (2, 3)
                            for jt in jts:
                                js = slice(jt * 128, (jt + 1) * 128)
                                nc.tensor.matmul(
                                    qs[jt][:, 0:H], t1[0][:, js], w2at[:],
                                    start=True, stop=False)
                                nc.tensor.matmul(
                                    qs[jt][:, 0:H], t1[1][:, js], w2bt[:],
                                    start=False, stop=False)
                            if with_bias:
                                for jt in jts:
                                    js = slice(jt * 128, (jt + 1) * 128)
                                    nc.tensor.matmul(
                                        qs[jt][:, 0:H], t1[2][0:kc, js],
                                        w2ct[0:kc, :],
                                        start=False, stop=True)
                            else:
                                # K=64 tails of the jt pair run concurrently
                                # in row groups (0,0) and (64,0)
                                jt0, jt1 = jts
                                js0 = slice(jt0 * 128, (jt0 + 1) * 128)
                                js1 = slice(jt1 * 128, (jt1 + 1) * 128)
                                nc.tensor.matmul(
                                    qs[jt0][:, 0:H], t1[2][0:64, js0],
                                    w2ct[0:64, :], start=False, stop=True)
                                nc.tensor.matmul(
                                    qs[jt1][:, 0:H], t1[2][64:128, js1],
                                    w2ct[64:128, :], start=False, stop=True)
                        return qs

                    def emit_l3(qs, i):
                        """L3 on DVE: fused relu*sign + row-sum per jt."""
                        for jt in range(NJT):
                            sv = scr.tile([128, H], F32, tag=f"scrV{jt % 2}")
                            nc.vector.scalar_tensor_tensor(
                                out=sv[:], in0=qs[jt][:, 0:H], scalar=0.0,
                                in1=sg[:], op0=ALU.max, op1=ALU.mult,
                                accum_out=osig[jt][:, i:i + 1])

                    # software pipeline: t1 one step ahead of the PE
                    t1_next = emit_t1(0)
                    prev = None
                    for i in range(NI):
                        t1 = t1_next
                        qs = emit_pe(t1, i)
                        if i + 1 < NI:
                            t1_next = emit_t1(i + 1)
                        if prev is not None:
                            emit_l3(*prev)
                        prev = (qs, i)
                    emit_l3(*prev)

                    # epilogue: store j-major [N, NI] slabs
                    for jt in range(NJT):
                        nc.sync.dma_start(
                            out[b, jt * 128:(jt + 1) * 128, :], osig[jt][:])

    nc.compile()
    return nc


def _prep(robot_embedding_tf, object_embedding_tf, z, W1, b1, W2, b2, W3, b3):
    """Host-side weight prep (O(H^2)) + per-core input maps."""
    f = np.float32
    bf = ml_dtypes.bfloat16
    robot = np.ascontiguousarray(robot_embedding_tf, dtype=f)
    obj = np.ascontiguousarray(object_embedding_tf, dtype=f)
    z = np.asarray(z, dtype=f)
    W1 = np.asarray(W1, dtype=f)
    b1 = np.asarray(b1, dtype=f)
    W2 = np.asarray(W2, dtype=f)
    b2 = np.asarray(b2, dtype=f)
    W3 = np.asarray(W3, dtype=f)
    b3 = np.asarray(b3, dtype=f)

    w3 = W3[:, 0]
    aw3 = np.abs(w3)
    with_bias = bool(np.any(b2))
    kc = 65 if with_bias else 64
    W2p = W2 * aw3[None, :]
    b2p = b2 * aw3
    W2cols = np.concatenate([W2p, b2p[None, :]], axis=0)  # [H+1, H]
    signs = np.ascontiguousarray(
        np.broadcast_to(np.sign(w3)[None, :], (128, H)), dtype=f)

    w2a_ = np.ascontiguousarray(W2cols[0:128], dtype=bf)
    w2b_ = np.ascontiguousarray(W2cols[128:256], dtype=bf)
    if with_bias:
        w2c_ = np.ascontiguousarray(W2cols[256:256 + kc], dtype=bf)
    else:
        w2c_ = np.ascontiguousarray(
            np.concatenate([W2cols[256:320], W2cols[256:320]], axis=0),
            dtype=bf)

    zA = z @ W1[E:D, :]                 # [B, H]
    zB = z @ W1[D + E:, :] + b1[None, :]
    zAT = np.ascontiguousarray(zA.T, dtype=f)
    zBT = np.ascontiguousarray(zB.T, dtype=f)
    W1A = np.ascontiguousarray(W1[0:E, :], dtype=f)
    W1B = np.ascontiguousarray(W1[D:D + E, :], dtype=f)

    shared = dict(obj=obj, W1A=W1A, W1B=W1B, zAT=zAT, zBT=zBT,
                  w2a=w2a_, w2b=w2b_, w2c=w2c_, signs=signs)
    in_maps = []
    for c in range(NCORES):
        m = dict(shared)
        m["robot"] = np.ascontiguousarray(robot[:, c * NI:(c + 1) * NI, :])
        in_maps.append(m)
    return in_maps, with_bias, float(b3[0])


def _run(trace=False, **inputs):
    in_maps, with_bias, b3v = _prep(**inputs)
    if with_bias not in _CACHE:
        _CACHE[with_bias] = _build(with_bias)
    nc = _CACHE[with_bias]
    res = bass_utils.run_bass_kernel_spmd(
        nc, in_maps, core_ids=list(range(NCORES)), trace=trace)
    dro = np.empty((B, N, N), dtype=np.float32)
    for c in range(NCORES):
        # device output is j-major [B, N, NI]; transpose to [B, NI, N]
        dro[:, c * NI:(c + 1) * NI, :] = np.transpose(
            res.results[c]["out"], (0, 2, 1))
    if b3v != 0.0:
        dro += b3v
    return dro, res


def kernel(**inputs) -> np.ndarray:
    dro, _ = _run(trace=False, **inputs)
    return dro
